# revision 1
# baseline (speedup 1.0000x reference)
"""Trainium2 Bass kernel for nn_DynaResidualBlock (hypernetwork residual block).

Reference computation (B=32, LAT=256, FIN=FOUT=32, FH=64, H=W=128):
    h  = lat @ W1 + b1                       # [B, 9408]
    ks = h @ W2 + b2                         # [B, 9408]  (W2 is 9408x9408 = 354 MB)
    per-sample 1x1 convs with kernels/biases sliced out of ks:
    x_s = k_short(x) ; y = k_out(lrelu(k_mid(lrelu(k_in(x))))) + x_s

Sharding over 8 cores:
  - hypernet contraction dim (9408) split 1176-per-core: core i holds
    W1[:, shard_i] and W2[shard_i, :] and computes a partial ks for ALL
    32 samples; per-segment AllToAll exchanges (overlapped with the W2
    streaming) + on-chip tree-sums hand core i the summed ks rows for
    its own 4 samples.
  - conv phase is data-parallel: core i processes samples 4i..4i+3, packed
    as 2 sample-pairs with block-diagonal weight matrices so each 1x1 conv
    over a 512-pixel tile is a single PE matmul.

Implementation notes:
  - Hypernet matmuls run as float32r (FP22 multiply, fp32 accumulate):
    single PE pass at full rate vs fp32's two half-rate passes. The conv
    phase runs fp16 (x is pre-cast host-side so its loads avoid the slow
    casting-DMA path).
  - W2's columns are permuted host-side so each generated conv kernel
    lands in SBUF already transposed into the PE's lhsT layout.
  - x gets 3 constant "ones" channels per pair so conv biases ride in as
    extra matmul rows (b_in, b_short, b_out fold into the matmuls).
  - lat is passed pre-transposed; b2 rides as an extra W2 row on core 0.
"""

import contextlib

import numpy as np

import concourse.bacc as bacc
import concourse.mybir as mybir
import concourse.tile as tile
from concourse.bass_utils import run_bass_kernel_spmd

N_CORES = 8
B, LAT, FIN, FOUT, FH, H, W = 32, 256, 32, 32, 64, 128, 128
HW = H * W
K_IN, K_MID, K_OUT, K_SH = FH * FIN, FH * FH, FOUT * FH, FOUT * FIN
K_TOT = K_IN + K_MID + K_OUT + K_SH + FH + FH + FOUT + FOUT  # 9408
SHARD = K_TOT // N_CORES  # 1176 hypernet columns per core
KP = SHARD + 1            # + one bias row (b2, on core 0 only)
KPAD = 1280               # h length padded to 10 chunks of 128
NCH = KPAD // 128         # 10
BPC = B // N_CORES        # 4 samples per core
XC = 2 * FIN + 3          # 67 = 2x32 x-channels + 3 ones-channels per pair
F32 = mybir.dt.float32
F32R = mybir.dt.float32r  # FP22 multiplies at full PE rate, fp32 accumulate
BF16 = mybir.dt.bfloat16


def _r(ap):
    return ap.bitcast(F32R)


OFF_IN, OFF_MID = 0, K_IN
OFF_OUT, OFF_SHC = K_IN + K_MID, K_IN + K_MID + K_OUT
OFF_B = OFF_SHC + K_SH  # 9216: b_in 64 | b_mid 64 | b_out 32 | b_short 32

# permuted ks layout: | biases 192 | k_inT 2048 | k_shortT 1024 | k_midT 4096
# | k_outT 2048 | — ordered so the AllToAll exchanges complete in the order
# phase B consumes them (k_out, the last conv stage, arrives last).
NB_IN, NB_MID, NB_OUT, NB_SH = 0, 64, 128, 160
NK_IN, NK_SH, NK_MID, NK_OUT = 192, 2240, 3264, 7360
# phase-A ks column groups (n-tiles of <=512 within each); k_out's group is
# computed FIRST so its exchange flies while W2 still streams, leaving k_mid
# as the only exchange exposed after phase A.
GROUPS = [(7360, 2048), (0, 2048), (2048, 1216), (3264, 2048), (5312, 2048)]
# three AllToAll exchanges; fired after the group that completes each span
A2A_SPEC = [(0, 3264), (3264, 4096), (7360, 2048)]
A2A_AFTER_GROUP = {0: 2, 2: 0, 4: 1}
FP16 = mybir.dt.float16

_CACHE: dict = {}


def _build():
    nc = bacc.Bacc("TRN2", target_bir_lowering=False, num_devices=N_CORES)
    AF = mybir.ActivationFunctionType

    xs = nc.dram_tensor("xs", [2, XC, HW], FP16, kind="ExternalInput")
    latT = nc.dram_tensor("latT", [LAT, B], F32, kind="ExternalInput")
    w1s = nc.dram_tensor("w1s", [LAT, KPAD], F32, kind="ExternalInput")
    b1s = nc.dram_tensor("b1s", [128, NCH], F32, kind="ExternalInput")
    w2s = nc.dram_tensor("w2s", [KP, K_TOT], F32, kind="ExternalInput")
    out = nc.dram_tensor("out", [2, 2 * FOUT, HW], F32, kind="ExternalOutput")

    zeros16 = nc.inline_tensor(np.zeros((128, 128), np.float16), name="zconst16")
    a2a_in = [nc.dram_tensor(f"a2a_in{g}", [B, w], F32)
              for g, (_, w) in enumerate(A2A_SPEC)]
    a2a_out = [nc.dram_tensor(f"a2a_out{g}", [B, w], F32)
               for g, (_, w) in enumerate(A2A_SPEC)]
    ks_own = [nc.dram_tensor(f"ks_own{g}", [BPC, w], F32)
              for g, (_, w) in enumerate(A2A_SPEC)]

    with tile.TileContext(nc) as tc, contextlib.ExitStack() as ctx:
        sing = ctx.enter_context(tc.tile_pool(name="sing", bufs=1))
        a2ap = ctx.enter_context(tc.tile_pool(name="a2ap", bufs=8))

        def a2a_fire(idx):
            nc.gpsimd.collective_compute(
                "AllToAll", mybir.AluOpType.bypass,
                replica_groups=[list(range(N_CORES))],
                ins=[a2a_in[idx][:, :].opt()], outs=[a2a_out[idx][:, :].opt()])

        def a2a_reduce(idx, eng):
            """Tree-sum the 8 chunks the AllToAll delivered; result lands in
            ks_own[idx]. The DMA engine must be one whose FIFO carries no
            traffic that the exchange would head-of-line block."""
            _, w = A2A_SPEC[idx]
            wf = BPC * w // 128
            ch = []
            for c in range(N_CORES):
                t = a2ap.tile([128, 128], F32, tag="a2c", name=f"a2c_{idx}_{c}")
                eng.dma_start(out=t[:, 0:wf],
                              in_=a2a_out[idx][BPC * c:BPC * (c + 1), :])
                ch.append(t)
            for i, j in [(0, 1), (2, 3), (4, 5), (6, 7), (0, 2), (4, 6), (0, 4)]:
                nc.vector.tensor_add(ch[i][:, 0:wf], ch[i][:, 0:wf], ch[j][:, 0:wf])
            eng.dma_start(out=ks_own[idx][:, :], in_=ch[0][:, 0:wf])

        # ---------------- Phase A: hypernet (partial ks for all B) --------
        with tc.tile_pool(name="psA", bufs=2, space="PSUM") as psA, \
             tc.tile_pool(name="psK", bufs=6, space="PSUM") as psK, \
             tc.tile_pool(name="w2pool", bufs=8) as w2pool, \
             tc.tile_pool(name="kspool", bufs=3) as kspool:
            latT_sb = sing.tile([128, 2 * B], F32R)
            w1_sb = sing.tile([128, 2 * KPAD], F32R)
            b1_sb = sing.tile([128, NCH], F32)
            hT_sb = sing.tile([128, NCH * B], F32R)
            for l in range(2):
                nc.sync.dma_start(out=latT_sb[:, l * B:(l + 1) * B],
                                  in_=_r(latT[128 * l:128 * (l + 1), :]))
                nc.sync.dma_start(out=w1_sb[:, l * KPAD:(l + 1) * KPAD],
                                  in_=_r(w1s[128 * l:128 * (l + 1), :]))
            nc.sync.dma_start(out=b1_sb[:, :], in_=b1s[:, :])

            # hT[kcol, b] = sum_l W1[l, kcol] * lat[b, l]  (+ b1[kcol])
            for c in range(NCH):
                ph = psA.tile([128, B], F32, tag="ph", name=f"ph{c}")
                for l in range(2):
                    nc.tensor.matmul(
                        ph,
                        lhsT=w1_sb[:, l * KPAD + 128 * c: l * KPAD + 128 * c + 128],
                        rhs=latT_sb[:, l * B:(l + 1) * B],
                        start=(l == 0), stop=(l == 1))
                nc.scalar.activation(out=hT_sb[:, c * B:(c + 1) * B], in_=ph,
                                     func=AF.Identity, bias=b1_sb[:, c:c + 1],
                                     scale=1.0)

            # partial ks[b, n] = sum_k h[b, k] * W2s[k, n] ; W2s streamed
            for g, (g0, gw) in enumerate(GROUPS):
                nts = [(t, min(512, gw - t)) for t in range(0, gw, 512)]
                pts = [psK.tile([B, 512], F32, tag="pk", name=f"pk_{g0}_{t0}")
                       for t0, _ in nts]
                for c in range(NCH):
                    kc = 128 if c < NCH - 1 else KP - 128 * (NCH - 1)
                    w2t = w2pool.tile([128, 2048], F32R, tag="w2t",
                                      name=f"w2t_{g0}_{c}")
                    dma_eng = nc.sync if c % 2 == 0 else nc.scalar
                    dma_eng.dma_start(out=w2t[0:kc, 0:gw],
                                      in_=_r(w2s[128 * c:128 * c + kc, g0:g0 + gw]))
                    for ti, (t0, tw) in enumerate(nts):
                        nc.tensor.matmul(
                            pts[ti][:, 0:tw],
                            lhsT=hT_sb[0:kc, c * B:(c + 1) * B],
                            rhs=w2t[0:kc, t0:t0 + tw],
                            start=(c == 0), stop=(c == NCH - 1))
                ks_sb = kspool.tile([B, 2048], F32, tag="ks_sb", name=f"ks_{g0}")
                for ti, (t0, tw) in enumerate(nts):
                    nc.scalar.activation(out=ks_sb[:, t0:t0 + tw],
                                         in_=pts[ti][:, 0:tw], func=AF.Copy)
                # store into the right A2A input tensor
                a_i = next(i for i, (r0, rw) in enumerate(A2A_SPEC)
                           if r0 <= g0 < r0 + rw)
                r0, _ = A2A_SPEC[a_i]
                nc.sync.dma_start(out=a2a_in[a_i][:, g0 - r0:g0 - r0 + gw],
                                  in_=ks_sb[:, 0:gw])
                if g in A2A_AFTER_GROUP:
                    idx = A2A_AFTER_GROUP[g]
                    a2a_fire(idx)
                    if idx != 1:
                        a2a_reduce(idx, nc.gpsimd)

        # k_mid's tree-sum rides the scalar queue: W2 streaming is done by
        # the time it can run, and gpsimd would serialize it behind the
        # phase-B weight loads.
        a2a_reduce(1, nc.scalar)

        koa, kob, koc = ks_own

        # ---------------- Phase B: per-sample 1x1 convs (fp16) ------------
        with tc.tile_pool(name="wts", bufs=1) as wts, \
             tc.tile_pool(name="ps1", bufs=2, space="PSUM") as ps1p, \
             tc.tile_pool(name="ps2", bufs=1, space="PSUM") as ps2p, \
             tc.tile_pool(name="ps3", bufs=1, space="PSUM") as ps3p, \
             tc.tile_pool(name="xin", bufs=5) as xinp, \
             tc.tile_pool(name="ys", bufs=3) as ysp, \
             tc.tile_pool(name="outp", bufs=3) as outp:
            pair_wts = []
            for p in range(2):
                sa = 2 * p
                L1 = wts.tile([XC, 128], FP16, name=f"L1_{p}")
                L2 = wts.tile([128, 128], FP16, name=f"L2_{p}")
                L3 = wts.tile([128, FH], FP16, name=f"L3_{p}")
                L4 = wts.tile([XC, FH], FP16, name=f"L4_{p}")
                bmid = wts.tile([128, 1], F32, name=f"bm_{p}")
                for t in (L1, L2, L3, L4):
                    pp, ff = t.shape
                    nc.gpsimd.dma_start(out=t, in_=zeros16[0:pp, 0:ff])
                for s in range(2):
                    row = sa + s
                    # conv kernels, already transposed to lhsT layout by the
                    # host-side W2 column permutation (fp32 -> fp16 cast DMA)
                    nc.gpsimd.dma_start(
                        out=L1[32 * s:32 * s + 32, 64 * s:64 * s + 64],
                        in_=koa[row, NK_IN:NK_IN + K_IN]
                        .rearrange("(i o) -> i o", i=FIN))
                    nc.gpsimd.dma_start(
                        out=L3[64 * s:64 * s + 64, 32 * s:32 * s + 32],
                        in_=koc[sa + s, :].rearrange("(i o) -> i o", i=FH))
                    nc.gpsimd.dma_start(
                        out=L4[32 * s:32 * s + 32, 32 * s:32 * s + 32],
                        in_=koa[row, NK_SH:NK_SH + K_SH]
                        .rearrange("(i o) -> i o", i=FIN))
                    # bias rows, consumed via the ones-channels of xs
                    nc.gpsimd.dma_start(out=L1[64 + s:65 + s, 64 * s:64 * s + 64],
                                        in_=koa[row, NB_IN:NB_IN + FH])
                    nc.gpsimd.dma_start(out=L4[64 + s:65 + s, 32 * s:32 * s + 32],
                                        in_=koa[row, NB_SH:NB_SH + FOUT])
                    nc.gpsimd.dma_start(out=L4[66:67, 32 * s:32 * s + 32],
                                        in_=koa[row, NB_OUT:NB_OUT + FOUT])
                nc.gpsimd.dma_start(out=bmid[:, 0:1],
                                    in_=koa[sa:sa + 2, NB_MID:NB_MID + FH])
                # L2 last: it waits on the final exchange and would otherwise
                # head-of-line block the loads above in the gpsimd FIFO
                for s in range(2):
                    nc.gpsimd.dma_start(
                        out=L2[64 * s:64 * s + 64, 64 * s:64 * s + 64],
                        in_=kob[sa + s, :].rearrange("(i o) -> i o", i=FH))
                pair_wts.append((L1, L2, L3, L4, bmid))

            (L1a, L2a, L3a, L4a, bma), (L1b, L2b, L3b, L4b, bmb) = pair_wts
            for jj in range(HW // 1024):
                c0 = 1024 * jj
                x0 = xinp.tile([XC, 1024], FP16, tag="x0", name=f"x0_{jj}")
                x1 = xinp.tile([XC, 1024], FP16, tag="x1", name=f"x1_{jj}")
                nc.sync.dma_start(out=x0, in_=xs[0, :, c0:c0 + 1024])
                nc.sync.dma_start(out=x1, in_=xs[1, :, c0:c0 + 1024])
                o0 = outp.tile([FH, 1024], F32, tag="o0", name=f"o0_{jj}")
                o1 = outp.tile([FH, 1024], F32, tag="o1", name=f"o1_{jj}")
                for h in range(2):
                    s0 = 512 * h
                    xv0, xv1 = x0[:, s0:s0 + 512], x1[:, s0:s0 + 512]
                    p1 = ps1p.tile([128, 1024], F32, tag="p1",
                                   name=f"p1_{jj}_{h}")
                    nc.tensor.matmul(p1[:, 0:512], lhsT=L1a, rhs=xv0,
                                     start=True, stop=True)
                    nc.tensor.matmul(p1[:, 512:1024], lhsT=L1b, rhs=xv1,
                                     start=True, stop=True)
                    y1 = ysp.tile([128, 1024], FP16, tag="y1",
                                  name=f"y1_{jj}_{h}")
                    nc.scalar.activation(out=y1, in_=p1, func=AF.Lrelu,
                                         bias=0.0, scale=1.0, alpha=0.01)
                    p2 = ps2p.tile([128, 1024], F32, tag="p2",
                                   name=f"p2_{jj}_{h}")
                    nc.tensor.matmul(p2[:, 0:512], lhsT=L2a, rhs=y1[:, 0:512],
                                     start=True, stop=True)
                    nc.tensor.matmul(p2[:, 512:1024], lhsT=L2b,
                                     rhs=y1[:, 512:1024], start=True, stop=True)
                    y2 = ysp.tile([128, 1024], FP16, tag="y2",
                                  name=f"y2_{jj}_{h}")
                    nc.scalar.activation(out=y2[:, 0:512], in_=p2[:, 0:512],
                                         func=AF.Lrelu, bias=bma[:, 0:1],
                                         scale=1.0, alpha=0.01)
                    nc.scalar.activation(out=y2[:, 512:1024], in_=p2[:, 512:1024],
                                         func=AF.Lrelu, bias=bmb[:, 0:1],
                                         scale=1.0, alpha=0.01)
                    p3 = ps3p.tile([FH, 1024], F32, tag="p3",
                                   name=f"p3_{jj}_{h}")
                    nc.tensor.matmul(p3[:, 0:512], lhsT=L3a, rhs=y2[:, 0:512],
                                     start=True, stop=False)
                    nc.tensor.matmul(p3[:, 0:512], lhsT=L4a, rhs=xv0,
                                     start=False, stop=True)
                    nc.tensor.matmul(p3[:, 512:1024], lhsT=L3b,
                                     rhs=y2[:, 512:1024], start=True, stop=False)
                    nc.tensor.matmul(p3[:, 512:1024], lhsT=L4b, rhs=xv1,
                                     start=False, stop=True)
                    nc.vector.tensor_copy(o0[:, s0:s0 + 512], p3[:, 0:512])
                    nc.vector.tensor_copy(o1[:, s0:s0 + 512], p3[:, 512:1024])
                nc.gpsimd.dma_start(out=out[0, :, c0:c0 + 1024], in_=o0)
                nc.gpsimd.dma_start(out=out[1, :, c0:c0 + 1024], in_=o1)

    nc.compile()
    return nc


def _seg_perm(rows, cols):
    # new position (c, r) holds old flat index r*cols + c
    return np.arange(rows * cols).reshape(rows, cols).T.ravel()


def _perm():
    # permutation of ks columns: conv kernels arrive transposed (lhsT layout)
    # and segments are reordered to the phase-B consumption order
    return np.concatenate([
        np.arange(OFF_B, K_TOT),          # biases first
        OFF_IN + _seg_perm(FH, FIN),      # k_inT
        OFF_SHC + _seg_perm(FOUT, FIN),   # k_shortT
        OFF_MID + _seg_perm(FH, FH),      # k_midT
        OFF_OUT + _seg_perm(FOUT, FH),    # k_outT
    ])


def _prep_in_maps(x, lat, W1, b1, W2, b2):
    x = np.ascontiguousarray(x, np.float32)
    lat = np.ascontiguousarray(lat, np.float32)
    W1 = np.ascontiguousarray(W1, np.float32)
    b1 = np.asarray(b1, np.float32)
    W2 = np.asarray(W2, np.float32)
    b2 = np.asarray(b2, np.float32)

    perm = _perm()
    W2p = W2[:, perm]
    b2p = b2[perm]
    latT = np.ascontiguousarray(lat.T)
    xr = x.reshape(B, FIN, HW)

    in_maps = []
    for i in range(N_CORES):
        sh = slice(i * SHARD, (i + 1) * SHARD)
        w1p = np.zeros((LAT, KPAD), np.float32)
        w1p[:, :SHARD] = W1[:, sh]
        b1p = np.zeros((KPAD,), np.float32)
        b1p[:SHARD] = b1[sh]
        b1p[SHARD] = 1.0  # the "ones" h-slot that carries b2
        w2a = np.zeros((KP, K_TOT), np.float32)
        w2a[:SHARD] = W2p[sh]
        if i == 0:
            w2a[SHARD] = b2p
        xsi = np.ones((2, XC, HW), np.float16)
        for p in range(2):
            xsi[p, 0:FIN] = xr[4 * i + 2 * p]
            xsi[p, FIN:2 * FIN] = xr[4 * i + 2 * p + 1]
        in_maps.append({
            "xs": xsi,
            "latT": latT,
            "w1s": w1p,
            "b1s": np.ascontiguousarray(b1p.reshape(NCH, 128).T),
            "w2s": w2a,
        })
    return in_maps


def _run(in_maps, **kwargs):
    if "nc" not in _CACHE:
        _CACHE["nc"] = _build()
    return run_bass_kernel_spmd(_CACHE["nc"], in_maps,
                                core_ids=list(range(N_CORES)), **kwargs)


def _assemble(results):
    parts = [r["out"].reshape(BPC, FOUT, H, W) for r in results]
    return np.ascontiguousarray(np.concatenate(parts, axis=0))


def kernel(x, lat, W1, b1, W2, b2):
    in_maps = _prep_in_maps(x, lat, W1, b1, W2, b2)
    res = _run(in_maps)
    return _assemble(res.results)



# revision 12
# speedup vs baseline: 1.0104x; 1.0104x over previous
"""Trainium2 Bass kernel for nn_DynaResidualBlock (hypernetwork residual block).

Reference computation (B=32, LAT=256, FIN=FOUT=32, FH=64, H=W=128):
    h  = lat @ W1 + b1                       # [B, 9408]
    ks = h @ W2 + b2                         # [B, 9408]  (W2 is 9408x9408)
    per-sample 1x1 convs with kernels/biases sliced out of ks:
    x_s = k_short(x) ; y = k_out(lrelu(k_mid(lrelu(k_in(x))))) + x_s

Sharding over 8 cores (contraction-dim sharding of the hypernet):
  - core i holds W1[:, shard_i] / W2[shard_i, :] (1176 rows) in fp16 and
    computes a partial ks for ALL 32 samples; three ReduceScatters (one per
    ks column segment, fired as soon as that segment's columns finish
    streaming) sum the partials in-network and hand core i the final ks rows
    for its own 4 samples.  fp16 data movement halves the dominant W2 HBM
    stream vs fp32; partials/final ks stay fp16 (errors ~1e-3 << 2e-2 gate).
  - conv phase is data-parallel (core i owns samples 4i..4i+3, packed as two
    sample-pairs with block-diagonal weights) and runs STAGE-SEQUENTIAL:
    conv1 for all pixels as soon as k_in lands (overlapped with the W2
    stream of k_mid/k_out), then conv2, then conv3+shortcut.  Back-to-back
    matmuls per stage keep the PE pstate ramped; leaky-relu work is split
    across scalar/vector/gpsimd so no single engine paces a stage.
  - segment stream order [biases+k_in+k_short | k_mid | k_out] matches conv
    consumption order; each ReduceScatter pays a ~15us collective latency,
    so the last (k_out) is the smallest and its conv3 is the only stage
    exposed after the stream.
"""

import contextlib

import numpy as np

import concourse.bacc as bacc
import concourse.mybir as mybir
import concourse.tile as tile
from concourse.bass_utils import run_bass_kernel_spmd

N_CORES = 8
B, LAT, FIN, FOUT, FH, H, W = 32, 256, 32, 32, 64, 128, 128
HW = H * W
K_IN, K_MID, K_OUT, K_SH = FH * FIN, FH * FH, FOUT * FH, FOUT * FIN
K_TOT = K_IN + K_MID + K_OUT + K_SH + FH + FH + FOUT + FOUT  # 9408
SHARD = K_TOT // N_CORES  # 1176 hypernet contraction rows per core
KP = SHARD + 1            # + one bias row (b2, on core 0 only)
KPAD = 1280               # padded to 10 chunks of 128
NCH = KPAD // 128         # 10
BPC = B // N_CORES        # 4 samples per core
XC = 2 * FIN + 3          # 67 = 2x32 x-channels + 3 ones-channels per pair
F32 = mybir.dt.float32
FP16 = mybir.dt.float16

OFF_IN, OFF_MID = 0, K_IN
OFF_OUT, OFF_SHC = K_IN + K_MID, K_IN + K_MID + K_OUT
OFF_B = OFF_SHC + K_SH  # 9216: b_in 64 | b_mid 64 | b_out 32 | b_short 32

# permuted ks layout: | biases 192 | k_inT 2048 | k_shortT 1024 | k_midT 4096
# | k_outT 2048 |
NB_IN, NB_MID, NB_OUT, NB_SH = 0, 64, 128, 160
NK_IN, NK_SH, NK_MID, NK_OUT = 192, 2240, 3264, 7360
# column groups streamed in conv consumption order; ReduceScatter fired
# after the group that completes each segment's span.
GROUPS = [(0, 2048), (2048, 1216), (3264, 2048), (5312, 2048), (7360, 2048)]
RS_SPEC = [(0, 3264), (3264, 4096), (7360, 2048)]
RS_AFTER_GROUP = {1: 0, 3: 1, 4: 2}

_CACHE: dict = {}


def _build():
    nc = bacc.Bacc("TRN2", target_bir_lowering=False, num_devices=N_CORES)
    AF = mybir.ActivationFunctionType
    ALU = mybir.AluOpType

    xs = nc.dram_tensor("xs", [2, XC, HW], FP16, kind="ExternalInput")
    latT = nc.dram_tensor("latT", [LAT, B], FP16, kind="ExternalInput")
    w1s = nc.dram_tensor("w1s", [LAT, KPAD], FP16, kind="ExternalInput")
    b1s = nc.dram_tensor("b1s", [128, NCH], F32, kind="ExternalInput")
    w2s = nc.dram_tensor("w2s", [KP, K_TOT], FP16, kind="ExternalInput")
    out = nc.dram_tensor("out", [2, 2 * FOUT, HW], FP16, kind="ExternalOutput")

    zeros16 = nc.inline_tensor(np.zeros((128, 128), np.float16), name="zconst16")
    rs_in = [nc.dram_tensor(f"rs_in{g}", [B, w], FP16)
             for g, (_, w) in enumerate(RS_SPEC)]
    ks_own = [nc.dram_tensor(f"ks_own{g}", [BPC, w], FP16)
              for g, (_, w) in enumerate(RS_SPEC)]

    with tile.TileContext(nc) as tc, contextlib.ExitStack() as ctx:
        sing = ctx.enter_context(tc.tile_pool(name="sing", bufs=1))
        kspool = ctx.enter_context(tc.tile_pool(name="kspool", bufs=2))
        outp = ctx.enter_context(tc.tile_pool(name="outp", bufs=4))

        # ---------------- constants + hypernet first layer ----------------
        latT_sb = sing.tile([128, 2 * B], FP16)
        w1_sb = sing.tile([128, 2 * KPAD], FP16)
        b1_sb = sing.tile([128, NCH], F32)
        hT_sb = sing.tile([128, NCH * B], FP16)
        for l in range(2):
            nc.sync.dma_start(out=latT_sb[:, l * B:(l + 1) * B],
                              in_=latT[128 * l:128 * (l + 1), :])
            nc.sync.dma_start(out=w1_sb[:, l * KPAD:(l + 1) * KPAD],
                              in_=w1s[128 * l:128 * (l + 1), :])
        nc.sync.dma_start(out=b1_sb[:, :], in_=b1s[:, :])

        with tc.tile_pool(name="psA", bufs=2, space="PSUM") as psA:
            # hT[kcol, b] = sum_l W1[l, kcol] * lat[b, l]  (+ b1[kcol])
            for c in range(NCH):
                ph = psA.tile([128, B], F32, tag="ph", name=f"ph{c}")
                for l in range(2):
                    nc.tensor.matmul(
                        ph,
                        lhsT=w1_sb[:, l * KPAD + 128 * c: l * KPAD + 128 * c + 128],
                        rhs=latT_sb[:, l * B:(l + 1) * B],
                        start=(l == 0), stop=(l == 1))
                nc.scalar.activation(out=hT_sb[:, c * B:(c + 1) * B], in_=ph,
                                     func=AF.Identity, bias=b1_sb[:, c:c + 1],
                                     scale=1.0)

        # phase-B weight tiles (zero-filled now; loaded as segments land)
        pair_wts = []
        for p in range(2):
            L1 = sing.tile([XC, 128], FP16, name=f"L1_{p}")
            L2 = sing.tile([128, 128], FP16, name=f"L2_{p}")
            L3 = sing.tile([128, FH], FP16, name=f"L3_{p}")
            L4 = sing.tile([XC, FH], FP16, name=f"L4_{p}")
            bmid = sing.tile([128, 1], F32, name=f"bm_{p}")
            for t in (L1, L2, L3, L4):
                pp, ff = t.shape
                nc.gpsimd.dma_start(out=t, in_=zeros16[0:pp, 0:ff])
            pair_wts.append((L1, L2, L3, L4, bmid))

        x_sb = sing.tile([XC, 2 * HW], FP16, name="x_sb")
        y1 = [sing.tile([128, HW], FP16, name=f"y1_{p}") for p in range(2)]

        def rs_fire(idx):
            nc.gpsimd.collective_compute(
                "ReduceScatter", mybir.AluOpType.add,
                replica_groups=[list(range(N_CORES))],
                ins=[rs_in[idx][:, :].opt()], outs=[ks_own[idx][:, :].opt()])

        koa, kob, koc = ks_own

        def load_seg0_weights():
            """L1 (k_in), L4 (k_short + b_short + b_out), bmid — all live in
            ks_own[0].  gpsimd queue, placed after the RS0 fire in its FIFO."""
            for p in range(2):
                L1, L2, L3, L4, bmid = pair_wts[p]
                sa = 2 * p
                for s in range(2):
                    row = sa + s
                    nc.gpsimd.dma_start(
                        out=L1[32 * s:32 * s + 32, 64 * s:64 * s + 64],
                        in_=koa[row, NK_IN:NK_IN + K_IN]
                        .rearrange("(i o) -> i o", i=FIN))
                    nc.gpsimd.dma_start(
                        out=L4[32 * s:32 * s + 32, 32 * s:32 * s + 32],
                        in_=koa[row, NK_SH:NK_SH + K_SH]
                        .rearrange("(i o) -> i o", i=FIN))
                    nc.gpsimd.dma_start(out=L1[64 + s:65 + s, 64 * s:64 * s + 64],
                                        in_=koa[row, NB_IN:NB_IN + FH])
                    nc.gpsimd.dma_start(out=L4[64 + s:65 + s, 32 * s:32 * s + 32],
                                        in_=koa[row, NB_SH:NB_SH + FOUT])
                    nc.gpsimd.dma_start(out=L4[66:67, 32 * s:32 * s + 32],
                                        in_=koa[row, NB_OUT:NB_OUT + FOUT])
                nc.gpsimd.dma_start(out=bmid[:, 0:1],
                                    in_=koa[sa:sa + 2, NB_MID:NB_MID + FH])

        def load_seg1_weights():
            for p in range(2):
                L2 = pair_wts[p][1]
                for s in range(2):
                    nc.gpsimd.dma_start(
                        out=L2[64 * s:64 * s + 64, 64 * s:64 * s + 64],
                        in_=kob[2 * p + s, :].rearrange("(i o) -> i o", i=FH))

        def load_seg2_weights():
            for p in range(2):
                L3 = pair_wts[p][2]
                for s in range(2):
                    nc.gpsimd.dma_start(
                        out=L3[64 * s:64 * s + 64, 32 * s:32 * s + 32],
                        in_=koc[2 * p + s, :].rearrange("(i o) -> i o", i=FH))

        # ---------- Phase A: stream W2, partial ks, ReduceScatters --------
        # pcv co-opened with psK: conv1/conv2 overlap the W2 stream.
        with tc.tile_pool(name="psK", bufs=4, space="PSUM") as psK, \
             tc.tile_pool(name="pcv", bufs=2, space="PSUM") as pcv, \
             tc.tile_pool(name="w2pool", bufs=8) as w2pool:
            def finish_group(g):
                """PSUM -> fp16 SBUF (scalar) -> rs_in (gpsimd queue)."""
                g0, gw, nts, pts = group_state[g]
                ks_sb = kspool.tile([B, 2048], FP16, tag="ks_sb",
                                    name=f"ks_{g0}")
                for ti, (t0, tw) in enumerate(nts):
                    nc.scalar.activation(out=ks_sb[:, t0:t0 + tw],
                                         in_=pts[ti][:, 0:tw], func=AF.Copy)
                r_i = next(i for i, (r0, rw) in enumerate(RS_SPEC)
                           if r0 <= g0 < r0 + rw)
                r0, _ = RS_SPEC[r_i]
                nc.gpsimd.dma_start(out=rs_in[r_i][:, g0 - r0:g0 - r0 + gw],
                                    in_=ks_sb[:, 0:gw])

            group_state = {}
            for g, (g0, gw) in enumerate(GROUPS):
                nts = [(t, min(512, gw - t)) for t in range(0, gw, 512)]
                pts = [psK.tile([B, 512], F32, tag="pk", name=f"pk_{g0}_{t0}")
                       for t0, _ in nts]
                group_state[g] = (g0, gw, nts, pts)
                for c in range(NCH):
                    kc = 128 if c < NCH - 1 else KP - 128 * (NCH - 1)
                    w2t = w2pool.tile([128, 2048], FP16, tag="w2t",
                                      name=f"w2t_{g0}_{c}")
                    dma_eng = nc.sync if c % 2 == 0 else nc.scalar
                    dma_eng.dma_start(out=w2t[0:kc, 0:gw],
                                      in_=w2s[128 * c:128 * c + kc, g0:g0 + gw])
                    for ti, (t0, tw) in enumerate(nts):
                        nc.tensor.matmul(
                            pts[ti][:, 0:tw],
                            lhsT=hT_sb[0:kc, c * B:(c + 1) * B],
                            rhs=w2t[0:kc, t0:t0 + tw],
                            start=(c == 0), stop=(c == NCH - 1))
                # groups 3/4's copies are deferred into conv1's scalar slots
                # so the k_mid/k_out finishes don't head-of-line block
                # conv1's scalar lrelus (or vice versa).
                if g <= 2:
                    finish_group(g)
                if g == 1:
                    rs_fire(0)
                if g == 2:
                    # after RS0 is queued: k_in/k_short/bias tile loads ride
                    # the gpsimd FIFO behind the RS0 fire (its data dep).
                    load_seg0_weights()
                if g == 3:
                    # x preload rides the queues after k_mid's chunks so it
                    # doesn't delay the k_mid ReduceScatter.
                    nc.sync.dma_start(out=x_sb[:, 0:HW], in_=xs[0, :, :])
                    nc.scalar.dma_start(out=x_sb[:, HW:2 * HW], in_=xs[1, :, :])

            # ---------------- Phase B: conv1 then conv2 -------------------
            # lrelu split across scalar+vector (gpsimd cannot touch PSUM).
            # Scalar's conv1 slots also host the deferred k_mid/k_out group
            # finishes at the points where their PSUM accumulations land.
            def lrelu(eng, dst, src, bias=None):
                # DVE instructions may read at most one PSUM operand, so the
                # two-op forms route the second op through the SBUF copy.
                if eng is nc.scalar:
                    eng.activation(out=dst, in_=src, func=AF.Lrelu,
                                   bias=(0.0 if bias is None else bias),
                                   scale=1.0, alpha=0.01)
                elif bias is None:
                    eng.tensor_scalar_mul(dst, src, 0.01)
                    eng.tensor_tensor(dst, dst, src, op=ALU.max)
                else:
                    eng.tensor_scalar_add(dst, src, bias)
                    eng.scalar_tensor_tensor(dst, dst, 0.01, dst,
                                             op0=ALU.mult, op1=ALU.max)

            n_sc = 0
            for p in range(2):
                L1 = pair_wts[p][0]
                for t in range(HW // 1024):
                    c0 = 1024 * t
                    idx = 16 * p + t
                    p1 = pcv.tile([128, 1024], F32, tag="pcv",
                                  name=f"p1_{p}_{t}")
                    for h in range(2):
                        s0 = c0 + 512 * h
                        nc.tensor.matmul(p1[:, 512 * h:512 * h + 512], lhsT=L1,
                                         rhs=x_sb[:, p * HW + s0:p * HW + s0 + 512],
                                         start=True, stop=True)
                    eng = nc.scalar if idx % 8 >= 5 else nc.vector
                    lrelu(eng, y1[p][:, c0:c0 + 1024], p1)
                    if eng is nc.scalar:
                        n_sc += 1
                        if n_sc == 4:
                            finish_group(3)
                            rs_fire(1)
                        elif n_sc == 12:
                            finish_group(4)
                            rs_fire(2)
                            load_seg1_weights()

            for p in range(2):
                L2, bmid = pair_wts[p][1], pair_wts[p][4]
                for t in range(HW // 1024):
                    c0 = 1024 * t
                    p2 = pcv.tile([128, 1024], F32, tag="pcv",
                                  name=f"p2_{p}_{t}")
                    for h in range(2):
                        nc.tensor.matmul(
                            p2[:, 512 * h:512 * h + 512], lhsT=L2,
                            rhs=y1[p][:, c0 + 512 * h:c0 + 512 * h + 512],
                            start=True, stop=True)
                    eng = nc.scalar if t % 2 == 0 else nc.vector
                    lrelu(eng, y1[p][:, c0:c0 + 1024], p2, bias=bmid[:, 0:1])

        # L3 loads issued after conv2's gpsimd lrelus so those aren't stuck
        # in the gpsimd FIFO behind a wait on the k_out ReduceScatter.
        load_seg2_weights()

        # ---------------- Phase B: conv3 + shortcut -----------------------
        with tc.tile_pool(name="ps3", bufs=3, space="PSUM") as ps3p:
            c3_eng = [nc.scalar, nc.vector] * 16
            for p in range(2):
                L3, L4 = pair_wts[p][2], pair_wts[p][3]
                for t in range(HW // 1024):
                    c0 = 1024 * t
                    p3 = ps3p.tile([FH, 1024], F32, tag="p3",
                                   name=f"p3_{p}_{t}")
                    for h in range(2):
                        s0 = c0 + 512 * h
                        nc.tensor.matmul(p3[:, 512 * h:512 * h + 512], lhsT=L3,
                                         rhs=y1[p][:, s0:s0 + 512],
                                         start=True, stop=False)
                        nc.tensor.matmul(p3[:, 512 * h:512 * h + 512], lhsT=L4,
                                         rhs=x_sb[:, p * HW + s0:p * HW + s0 + 512],
                                         start=False, stop=True)
                    osb = outp.tile([FH, 1024], FP16, tag="osb",
                                    name=f"o_{p}_{t}")
                    eng = c3_eng[16 * p + t]
                    if eng is nc.scalar:
                        eng.activation(out=osb, in_=p3, func=AF.Copy)
                    else:
                        eng.tensor_copy(osb, p3)
                    dq = nc.sync if t % 2 == 0 else nc.scalar
                    dq.dma_start(out=out[p, :, c0:c0 + 1024], in_=osb)

    nc.compile()
    return nc


def _seg_perm(rows, cols):
    # new position (c, r) holds old flat index r*cols + c
    return np.arange(rows * cols).reshape(rows, cols).T.ravel()


def _perm():
    # permutation of ks columns: conv kernels arrive transposed (lhsT layout)
    # and segments are reordered to the phase-B consumption order
    return np.concatenate([
        np.arange(OFF_B, K_TOT),          # biases first
        OFF_IN + _seg_perm(FH, FIN),      # k_inT
        OFF_SHC + _seg_perm(FOUT, FIN),   # k_shortT
        OFF_MID + _seg_perm(FH, FH),      # k_midT
        OFF_OUT + _seg_perm(FOUT, FH),    # k_outT
    ])


def _prep_in_maps(x, lat, W1, b1, W2, b2):
    x = np.asarray(x, np.float32)
    lat = np.asarray(lat, np.float32)
    W1 = np.asarray(W1, np.float32)
    b1 = np.asarray(b1, np.float32)
    W2 = np.asarray(W2, np.float32)
    b2 = np.asarray(b2, np.float32)

    perm = _perm()
    W2p = np.ascontiguousarray(W2[:, perm].astype(np.float16))
    b2p = b2[perm].astype(np.float16)
    latT = np.ascontiguousarray(lat.T.astype(np.float16))
    xr = x.reshape(B, FIN, HW)

    in_maps = []
    for i in range(N_CORES):
        sh = slice(i * SHARD, (i + 1) * SHARD)
        w1p = np.zeros((LAT, KPAD), np.float16)
        w1p[:, :SHARD] = W1[:, sh]
        b1p = np.zeros((KPAD,), np.float32)
        b1p[:SHARD] = b1[sh]
        b1p[SHARD] = 1.0  # the "ones" h-slot that carries b2
        w2a = np.zeros((KP, K_TOT), np.float16)
        w2a[:SHARD] = W2p[sh]
        if i == 0:
            w2a[SHARD] = b2p
        xsi = np.ones((2, XC, HW), np.float16)
        for p in range(2):
            xsi[p, 0:FIN] = xr[4 * i + 2 * p]
            xsi[p, FIN:2 * FIN] = xr[4 * i + 2 * p + 1]
        in_maps.append({
            "xs": xsi,
            "latT": latT,
            "w1s": w1p,
            "b1s": np.ascontiguousarray(b1p.reshape(NCH, 128).T),
            "w2s": w2a,
        })
    return in_maps


def _run(in_maps, **kwargs):
    if "nc" not in _CACHE:
        _CACHE["nc"] = _build()
    return run_bass_kernel_spmd(_CACHE["nc"], in_maps,
                                core_ids=list(range(N_CORES)), **kwargs)


def _assemble(results):
    parts = [r["out"].astype(np.float32).reshape(BPC, FOUT, H, W)
             for r in results]
    return np.ascontiguousarray(np.concatenate(parts, axis=0))


def kernel(x, lat, W1, b1, W2, b2):
    in_maps = _prep_in_maps(x, lat, W1, b1, W2, b2)
    res = _run(in_maps)
    return _assemble(res.results)


# revision 17
# speedup vs baseline: 1.0530x; 1.0422x over previous
"""Trainium2 Bass kernel for nn_DynaResidualBlock (hypernetwork residual block).

Reference computation (B=32, LAT=256, FIN=FOUT=32, FH=64, H=W=128):
    h  = lat @ W1 + b1                       # [B, 9408]
    ks = h @ W2 + b2                         # [B, 9408]  (W2 is 9408x9408)
    per-sample 1x1 convs with kernels/biases sliced out of ks:
    x_s = k_short(x) ; y = k_out(lrelu(k_mid(lrelu(k_in(x))))) + x_s

Sharding over 8 cores (contraction-dim sharding of the hypernet):
  - core i holds W1[:, shard_i] / W2[shard_i, :] (1176 rows) in fp16 and
    computes a partial ks for ALL 32 samples; three ReduceScatters (one per
    ks column segment, fired as soon as that segment's columns finish
    streaming) sum the partials in-network and hand core i the final ks rows
    for its own 4 samples.  fp16 data movement halves the dominant W2 HBM
    stream vs fp32; partials/final ks stay fp16 (errors ~1e-3 << 2e-2 gate).
  - conv phase is data-parallel (core i owns samples 4i..4i+3, packed as two
    sample-pairs with block-diagonal weights) and runs STAGE-SEQUENTIAL:
    conv1 for all pixels as soon as k_in lands (overlapped with the W2
    stream of k_mid/k_out), then conv2, then conv3+shortcut.  Back-to-back
    matmuls per stage keep the PE pstate ramped; leaky-relu work is split
    across scalar/vector/gpsimd so no single engine paces a stage.
  - segment stream order [biases+k_in+k_short | k_mid | k_out] matches conv
    consumption order; each ReduceScatter pays a ~15us collective latency,
    so the last (k_out) is the smallest and its conv3 is the only stage
    exposed after the stream.
"""

import contextlib

import numpy as np

import concourse.bacc as bacc
import concourse.mybir as mybir
import concourse.tile as tile
from concourse.bass_utils import run_bass_kernel_spmd

N_CORES = 8
B, LAT, FIN, FOUT, FH, H, W = 32, 256, 32, 32, 64, 128, 128
HW = H * W
K_IN, K_MID, K_OUT, K_SH = FH * FIN, FH * FH, FOUT * FH, FOUT * FIN
K_TOT = K_IN + K_MID + K_OUT + K_SH + FH + FH + FOUT + FOUT  # 9408
SHARD = K_TOT // N_CORES  # 1176 hypernet contraction rows per core
KP = SHARD + 1            # + one bias row (b2, on core 0 only)
KPAD = 1280               # padded to 10 chunks of 128
NCH = KPAD // 128         # 10
BPC = B // N_CORES        # 4 samples per core
XC = 2 * FIN + 3          # 67 = 2x32 x-channels + 3 ones-channels per pair
F32 = mybir.dt.float32
FP16 = mybir.dt.float16

OFF_IN, OFF_MID = 0, K_IN
OFF_OUT, OFF_SHC = K_IN + K_MID, K_IN + K_MID + K_OUT
OFF_B = OFF_SHC + K_SH  # 9216: b_in 64 | b_mid 64 | b_out 32 | b_short 32

# permuted ks layout: | biases 192 | k_inT 2048 | k_shortT 1024 | k_midT 4096
# | k_outT 2048 |
NB_IN, NB_MID, NB_OUT, NB_SH = 0, 64, 128, 160
NK_IN, NK_SH, NK_MID, NK_OUT = 192, 2240, 3264, 7360
# column groups streamed in conv consumption order; ReduceScatter fired
# after the group that completes each segment's span.
GROUPS = [(0, 2048), (2048, 1216), (3264, 2048), (5312, 2048), (7360, 2048)]
RS_SPEC = [(0, 3264), (3264, 4096), (7360, 2048)]
RS_AFTER_GROUP = {1: 0, 3: 1, 4: 2}

_CACHE: dict = {}


def _build():
    nc = bacc.Bacc("TRN2", target_bir_lowering=False, num_devices=N_CORES)
    AF = mybir.ActivationFunctionType
    ALU = mybir.AluOpType

    xs = nc.dram_tensor("xs", [2, XC, HW], FP16, kind="ExternalInput")
    latT = nc.dram_tensor("latT", [LAT, B], FP16, kind="ExternalInput")
    w1s = nc.dram_tensor("w1s", [LAT, KPAD], FP16, kind="ExternalInput")
    b1s = nc.dram_tensor("b1s", [128, NCH], F32, kind="ExternalInput")
    w2s = nc.dram_tensor("w2s", [KP, K_TOT], FP16, kind="ExternalInput")
    out = nc.dram_tensor("out", [2, 2 * FOUT, HW], FP16, kind="ExternalOutput")

    zeros16 = nc.inline_tensor(np.zeros((128, 128), np.float16), name="zconst16")
    rs_in = [nc.dram_tensor(f"rs_in{g}", [B, w], FP16)
             for g, (_, w) in enumerate(RS_SPEC)]
    ks_own = [nc.dram_tensor(f"ks_own{g}", [BPC, w], FP16)
              for g, (_, w) in enumerate(RS_SPEC)]

    with tile.TileContext(nc) as tc, contextlib.ExitStack() as ctx:
        sing = ctx.enter_context(tc.tile_pool(name="sing", bufs=1))
        kspool = ctx.enter_context(tc.tile_pool(name="kspool", bufs=2))
        outp = ctx.enter_context(tc.tile_pool(name="outp", bufs=4))

        # ---------------- constants + hypernet first layer ----------------
        latT_sb = sing.tile([128, 2 * B], FP16)
        w1_sb = sing.tile([128, 2 * KPAD], FP16)
        b1_sb = sing.tile([128, NCH], F32)
        hT_sb = sing.tile([128, NCH * B], FP16)
        for l in range(2):
            nc.sync.dma_start(out=latT_sb[:, l * B:(l + 1) * B],
                              in_=latT[128 * l:128 * (l + 1), :])
            nc.sync.dma_start(out=w1_sb[:, l * KPAD:(l + 1) * KPAD],
                              in_=w1s[128 * l:128 * (l + 1), :])
        nc.sync.dma_start(out=b1_sb[:, :], in_=b1s[:, :])

        with tc.tile_pool(name="psA", bufs=2, space="PSUM") as psA:
            # hT[kcol, b] = sum_l W1[l, kcol] * lat[b, l]  (+ b1[kcol])
            for c in range(NCH):
                ph = psA.tile([128, B], F32, tag="ph", name=f"ph{c}")
                for l in range(2):
                    nc.tensor.matmul(
                        ph,
                        lhsT=w1_sb[:, l * KPAD + 128 * c: l * KPAD + 128 * c + 128],
                        rhs=latT_sb[:, l * B:(l + 1) * B],
                        start=(l == 0), stop=(l == 1))
                nc.scalar.activation(out=hT_sb[:, c * B:(c + 1) * B], in_=ph,
                                     func=AF.Identity, bias=b1_sb[:, c:c + 1],
                                     scale=1.0)

        # phase-B weight tiles (zero-filled now; loaded as segments land)
        pair_wts = []
        for p in range(2):
            L1 = sing.tile([XC, 128], FP16, name=f"L1_{p}")
            L2 = sing.tile([128, 128], FP16, name=f"L2_{p}")
            L3 = sing.tile([128, FH], FP16, name=f"L3_{p}")
            L4 = sing.tile([XC, FH], FP16, name=f"L4_{p}")
            bmid = sing.tile([128, 1], F32, name=f"bm_{p}")
            for t in (L1, L2, L3, L4):
                pp, ff = t.shape
                nc.gpsimd.dma_start(out=t, in_=zeros16[0:pp, 0:ff])
            pair_wts.append((L1, L2, L3, L4, bmid))

        x_sb = sing.tile([XC, 2 * HW], FP16, name="x_sb")
        y1 = [sing.tile([128, HW], FP16, name=f"y1_{p}") for p in range(2)]

        def rs_fire(idx):
            nc.gpsimd.collective_compute(
                "ReduceScatter", mybir.AluOpType.add,
                replica_groups=[list(range(N_CORES))],
                ins=[rs_in[idx][:, :].opt()], outs=[ks_own[idx][:, :].opt()])

        koa, kob, koc = ks_own

        def load_seg0_weights():
            """L1 (k_in), L4 (k_short + b_short + b_out), bmid — all live in
            ks_own[0].  gpsimd queue, placed after the RS0 fire in its FIFO."""
            for p in range(2):
                L1, L2, L3, L4, bmid = pair_wts[p]
                sa = 2 * p
                for s in range(2):
                    row = sa + s
                    nc.gpsimd.dma_start(
                        out=L1[32 * s:32 * s + 32, 64 * s:64 * s + 64],
                        in_=koa[row, NK_IN:NK_IN + K_IN]
                        .rearrange("(i o) -> i o", i=FIN))
                    nc.gpsimd.dma_start(
                        out=L4[32 * s:32 * s + 32, 32 * s:32 * s + 32],
                        in_=koa[row, NK_SH:NK_SH + K_SH]
                        .rearrange("(i o) -> i o", i=FIN))
                    nc.gpsimd.dma_start(out=L1[64 + s:65 + s, 64 * s:64 * s + 64],
                                        in_=koa[row, NB_IN:NB_IN + FH])
                    nc.gpsimd.dma_start(out=L4[64 + s:65 + s, 32 * s:32 * s + 32],
                                        in_=koa[row, NB_SH:NB_SH + FOUT])
                    nc.gpsimd.dma_start(out=L4[66:67, 32 * s:32 * s + 32],
                                        in_=koa[row, NB_OUT:NB_OUT + FOUT])
                nc.gpsimd.dma_start(out=bmid[:, 0:1],
                                    in_=koa[sa:sa + 2, NB_MID:NB_MID + FH])

        def load_seg1_weights():
            for p in range(2):
                L2 = pair_wts[p][1]
                for s in range(2):
                    nc.gpsimd.dma_start(
                        out=L2[64 * s:64 * s + 64, 64 * s:64 * s + 64],
                        in_=kob[2 * p + s, :].rearrange("(i o) -> i o", i=FH))

        def load_seg2_weights():
            for p in range(2):
                L3 = pair_wts[p][2]
                for s in range(2):
                    nc.gpsimd.dma_start(
                        out=L3[64 * s:64 * s + 64, 32 * s:32 * s + 32],
                        in_=koc[2 * p + s, :].rearrange("(i o) -> i o", i=FH))

        # ---------- Phase A: stream W2, partial ks, ReduceScatters --------
        # pcv co-opened with psK: conv1/conv2 overlap the W2 stream.
        with tc.tile_pool(name="psK", bufs=6, space="PSUM") as psK, \
             tc.tile_pool(name="w2pool", bufs=8) as w2pool:
            def finish_group(g):
                """PSUM -> fp16 SBUF (scalar) -> rs_in (gpsimd queue)."""
                g0, gw, nts, pts = group_state[g]
                ks_sb = kspool.tile([B, 2048], FP16, tag="ks_sb",
                                    name=f"ks_{g0}")
                for ti, (t0, tw) in enumerate(nts):
                    nc.scalar.activation(out=ks_sb[:, t0:t0 + tw],
                                         in_=pts[ti][:, 0:tw], func=AF.Copy)
                r_i = next(i for i, (r0, rw) in enumerate(RS_SPEC)
                           if r0 <= g0 < r0 + rw)
                r0, _ = RS_SPEC[r_i]
                nc.gpsimd.dma_start(out=rs_in[r_i][:, g0 - r0:g0 - r0 + gw],
                                    in_=ks_sb[:, 0:gw])

            group_state = {}
            for g, (g0, gw) in enumerate(GROUPS):
                nts = [(t, min(512, gw - t)) for t in range(0, gw, 512)]
                pts = [psK.tile([B, 512], F32, tag="pk", name=f"pk_{g0}_{t0}")
                       for t0, _ in nts]
                group_state[g] = (g0, gw, nts, pts)
                for c in range(NCH):
                    kc = 128 if c < NCH - 1 else KP - 128 * (NCH - 1)
                    w2t = w2pool.tile([128, 2048], FP16, tag="w2t",
                                      name=f"w2t_{g0}_{c}")
                    dma_eng = nc.sync if c % 2 == 0 else nc.scalar
                    dma_eng.dma_start(out=w2t[0:kc, 0:gw],
                                      in_=w2s[128 * c:128 * c + kc, g0:g0 + gw])
                    for ti, (t0, tw) in enumerate(nts):
                        nc.tensor.matmul(
                            pts[ti][:, 0:tw],
                            lhsT=hT_sb[0:kc, c * B:(c + 1) * B],
                            rhs=w2t[0:kc, t0:t0 + tw],
                            start=(c == 0), stop=(c == NCH - 1))
                finish_group(g)
                if g == 1:
                    rs_fire(0)
                if g == 2:
                    # after RS0 is queued: k_in/k_short/bias tile loads ride
                    # the gpsimd FIFO behind the RS0 fire (its data dep).
                    load_seg0_weights()
                if g == 3:
                    rs_fire(1)
                    load_seg1_weights()
                    # x preload rides the queues after k_mid's chunks so it
                    # doesn't delay the k_mid ReduceScatter.
                    nc.sync.dma_start(out=x_sb[:, 0:HW], in_=xs[0, :, :])
                    nc.scalar.dma_start(out=x_sb[:, HW:2 * HW], in_=xs[1, :, :])
                if g == 4:
                    rs_fire(2)
                    load_seg2_weights()

        # ---------------- Phase B: conv1 then conv2 -----------------------
        # lrelu split across scalar+vector (gpsimd cannot touch PSUM).
        with tc.tile_pool(name="pcv", bufs=3, space="PSUM") as pcv:
            def lrelu(eng, dst, src, bias=None):
                # DVE instructions may read at most one PSUM operand, so the
                # two-op forms route the second op through the SBUF copy.
                if eng is nc.scalar:
                    eng.activation(out=dst, in_=src, func=AF.Lrelu,
                                   bias=(0.0 if bias is None else bias),
                                   scale=1.0, alpha=0.01)
                elif bias is None:
                    eng.tensor_scalar_mul(dst, src, 0.01)
                    eng.tensor_tensor(dst, dst, src, op=ALU.max)
                else:
                    eng.tensor_scalar_add(dst, src, bias)
                    eng.scalar_tensor_tensor(dst, dst, 0.01, dst,
                                             op0=ALU.mult, op1=ALU.max)

            for p in range(2):
                L1 = pair_wts[p][0]
                for t in range(HW // 1024):
                    c0 = 1024 * t
                    idx = 16 * p + t
                    p1 = pcv.tile([128, 1024], F32, tag="pcv",
                                  name=f"p1_{p}_{t}")
                    for h in range(2):
                        s0 = c0 + 512 * h
                        nc.tensor.matmul(p1[:, 512 * h:512 * h + 512], lhsT=L1,
                                         rhs=x_sb[:, p * HW + s0:p * HW + s0 + 512],
                                         start=True, stop=True)
                    eng = nc.scalar if idx % 8 >= 5 else nc.vector
                    lrelu(eng, y1[p][:, c0:c0 + 1024], p1)

            for p in range(2):
                L2, bmid = pair_wts[p][1], pair_wts[p][4]
                for t in range(HW // 1024):
                    c0 = 1024 * t
                    p2 = pcv.tile([128, 1024], F32, tag="pcv",
                                  name=f"p2_{p}_{t}")
                    for h in range(2):
                        nc.tensor.matmul(
                            p2[:, 512 * h:512 * h + 512], lhsT=L2,
                            rhs=y1[p][:, c0 + 512 * h:c0 + 512 * h + 512],
                            start=True, stop=True)
                    eng = nc.scalar if t % 2 == 0 else nc.vector
                    lrelu(eng, y1[p][:, c0:c0 + 1024], p2, bias=bmid[:, 0:1])

        # ---------------- Phase B: conv3 + shortcut -----------------------
        with tc.tile_pool(name="ps3", bufs=3, space="PSUM") as ps3p:
            c3_eng = [nc.scalar, nc.vector] * 16
            for p in range(2):
                L3, L4 = pair_wts[p][2], pair_wts[p][3]
                for t in range(HW // 1024):
                    c0 = 1024 * t
                    p3 = ps3p.tile([FH, 1024], F32, tag="p3",
                                   name=f"p3_{p}_{t}")
                    for h in range(2):
                        s0 = c0 + 512 * h
                        nc.tensor.matmul(p3[:, 512 * h:512 * h + 512], lhsT=L3,
                                         rhs=y1[p][:, s0:s0 + 512],
                                         start=True, stop=False)
                        nc.tensor.matmul(p3[:, 512 * h:512 * h + 512], lhsT=L4,
                                         rhs=x_sb[:, p * HW + s0:p * HW + s0 + 512],
                                         start=False, stop=True)
                    osb = outp.tile([FH, 1024], FP16, tag="osb",
                                    name=f"o_{p}_{t}")
                    eng = c3_eng[16 * p + t]
                    if eng is nc.scalar:
                        eng.activation(out=osb, in_=p3, func=AF.Copy)
                    else:
                        eng.tensor_copy(osb, p3)
                    dq = nc.sync if t % 2 == 0 else nc.scalar
                    dq.dma_start(out=out[p, :, c0:c0 + 1024], in_=osb)

    nc.compile()
    return nc


def _seg_perm(rows, cols):
    # new position (c, r) holds old flat index r*cols + c
    return np.arange(rows * cols).reshape(rows, cols).T.ravel()


def _perm():
    # permutation of ks columns: conv kernels arrive transposed (lhsT layout)
    # and segments are reordered to the phase-B consumption order
    return np.concatenate([
        np.arange(OFF_B, K_TOT),          # biases first
        OFF_IN + _seg_perm(FH, FIN),      # k_inT
        OFF_SHC + _seg_perm(FOUT, FIN),   # k_shortT
        OFF_MID + _seg_perm(FH, FH),      # k_midT
        OFF_OUT + _seg_perm(FOUT, FH),    # k_outT
    ])


def _prep_in_maps(x, lat, W1, b1, W2, b2):
    x = np.asarray(x, np.float32)
    lat = np.asarray(lat, np.float32)
    W1 = np.asarray(W1, np.float32)
    b1 = np.asarray(b1, np.float32)
    W2 = np.asarray(W2, np.float32)
    b2 = np.asarray(b2, np.float32)

    perm = _perm()
    W2p = np.ascontiguousarray(W2[:, perm].astype(np.float16))
    b2p = b2[perm].astype(np.float16)
    latT = np.ascontiguousarray(lat.T.astype(np.float16))
    xr = x.reshape(B, FIN, HW)

    in_maps = []
    for i in range(N_CORES):
        sh = slice(i * SHARD, (i + 1) * SHARD)
        w1p = np.zeros((LAT, KPAD), np.float16)
        w1p[:, :SHARD] = W1[:, sh]
        b1p = np.zeros((KPAD,), np.float32)
        b1p[:SHARD] = b1[sh]
        b1p[SHARD] = 1.0  # the "ones" h-slot that carries b2
        w2a = np.zeros((KP, K_TOT), np.float16)
        w2a[:SHARD] = W2p[sh]
        if i == 0:
            w2a[SHARD] = b2p
        xsi = np.ones((2, XC, HW), np.float16)
        for p in range(2):
            xsi[p, 0:FIN] = xr[4 * i + 2 * p]
            xsi[p, FIN:2 * FIN] = xr[4 * i + 2 * p + 1]
        in_maps.append({
            "xs": xsi,
            "latT": latT,
            "w1s": w1p,
            "b1s": np.ascontiguousarray(b1p.reshape(NCH, 128).T),
            "w2s": w2a,
        })
    return in_maps


def _run(in_maps, **kwargs):
    if "nc" not in _CACHE:
        _CACHE["nc"] = _build()
    return run_bass_kernel_spmd(_CACHE["nc"], in_maps,
                                core_ids=list(range(N_CORES)), **kwargs)


def _assemble(results):
    parts = [r["out"].astype(np.float32).reshape(BPC, FOUT, H, W)
             for r in results]
    return np.ascontiguousarray(np.concatenate(parts, axis=0))


def kernel(x, lat, W1, b1, W2, b2):
    in_maps = _prep_in_maps(x, lat, W1, b1, W2, b2)
    res = _run(in_maps)
    return _assemble(res.results)
